# revision 12
# baseline (speedup 1.0000x reference)
"""Trainium2 Bass kernel for IR-Net style binarized conv block.

Computation (matches the reference nn.Module):
  1. Per-out-channel weight standardization -> sign -> {-1,+1}, power-of-2
     per-channel scale sw (host-side numpy; weights are tiny).
  2. ba = sign(x) (device, ScalarE Sign activation, exact in bf16/fp8).
  3. y = conv2d(ba, sign_w) * sw  -- 3x3, pad 1, stride 1. Done as 9 shifted
     matmuls over a zero-padded SBUF activation image, channels on the
     partition axis, accumulating in PSUM. Exact: products are +-1 summed in
     fp32 PSUM.
  4. Training-mode BatchNorm over the FULL batch: per-channel sum / sumsq are
     accumulated on-device (activation accum_out / tensor_tensor_reduce),
     AllReduce'd across the 8 cores (2KB), folded together with sw, gamma,
     beta into per-channel affine a*z + b.
  5. Hardtanh clip via tensor_scalar(min,max).

Sharding: pure data parallel, batch 32 -> 4 images per core x 8 cores.
"""

import numpy as np
import ml_dtypes

import concourse.bacc as bacc
import concourse.bass as bass
import concourse.tile as tile
from concourse import mybir
from concourse.bass_utils import run_bass_kernel_spmd

F32 = mybir.dt.float32
BF16 = mybir.dt.bfloat16
FP8 = mybir.dt.float8e4

P = 128          # SBUF partitions
CG = 2           # channel groups: 256 channels = 2 x 128
C = 256
BN_EPS = 1e-5
N_CORES = 8
WPAD = 64        # padded row length (w + 2 <= 64; multiple of 16 for fp8 APs)
RT = 8           # output rows per PSUM tile (8 * 56 = 448 <= 512 fp32/bank)


def build_kernel(b_per_core=4, h=56, w=56, n_cores=N_CORES, use_fp8=True):
    """Build the per-core Bass program. Returns the compiled Bacc instance."""
    # fp8 DoubleRow needs a flat 3D moving AP [K, 2, N]; row-blocks are taken
    # as contiguous 512-elem (8 x 64) slices of the padded image, which can
    # overrun the last padded row by up to kh*WPAD+kw -- give each image 2
    # spare zero rows at the bottom.
    HP = h + 2 + (2 if use_fp8 else 0)
    assert w + 2 <= WPAD
    assert h % RT == 0
    tiles_per_img = h // RT
    NT = b_per_core * tiles_per_img     # PSUM tiles per output-channel group
    FREE = RT * w                       # moving free dim per matmul
    nhw_total = n_cores * b_per_core * h * w
    adt = FP8 if use_fp8 else BF16

    nc = bacc.Bacc(
        "TRN2", target_bir_lowering=False, debug=False, num_devices=n_cores
    )
    x_d = nc.dram_tensor("x", [b_per_core, C, h, w], F32, kind="ExternalInput").ap()
    w_d = nc.dram_tensor("wsgn", [P, CG, 9, C], adt, kind="ExternalInput").ap()
    coef_d = nc.dram_tensor("coef", [P, CG, 3], F32, kind="ExternalInput").ap()
    out_d = nc.dram_tensor(
        "out", [b_per_core, C, h, w], F32, kind="ExternalOutput"
    ).ap()

    mult = mybir.AluOpType.mult
    add = mybir.AluOpType.add
    subtract = mybir.AluOpType.subtract
    amin = mybir.AluOpType.min
    amax = mybir.AluOpType.max
    AF = mybir.ActivationFunctionType

    with tile.TileContext(nc) as tc:
        with (
            tc.tile_pool(name="singles", bufs=1) as singles,
            tc.tile_pool(name="xs", bufs=2) as xs_pool,
            tc.tile_pool(name="psum", bufs=8, space="PSUM") as psum_pool,
            tc.tile_pool(name="sq", bufs=2) as sq_pool,
            tc.tile_pool(name="stage", bufs=3) as stage_pool,
            tc.tile_pool(name="small", bufs=1) as small,
            tc.tile_pool(name="dram", bufs=1, space="DRAM") as dram,
        ):
            # ---- constants ----
            wsb = singles.tile([P, CG, 9, C], adt)
            nc.sync.dma_start(out=wsb[:], in_=w_d)
            coef = singles.tile([P, CG, 3], F32)
            nc.sync.dma_start(out=coef[:], in_=coef_d)

            # ---- padded, binarized activations (resident) ----
            acts = singles.tile([P, CG, b_per_core, HP, WPAD], adt)
            # zero borders (cols >= w+1 are never read except col w+1 itself;
            # memset the whole right margin anyway -- it's cheap and safe)
            nc.vector.memset(acts[:, :, :, 0, :], 0.0)
            nc.vector.memset(acts[:, :, :, h + 1 : HP, :], 0.0)
            nc.vector.memset(acts[:, :, :, :, 0:1], 0.0)
            nc.vector.memset(acts[:, :, :, :, w + 1 : WPAD], 0.0)

            # DMA x in per (image, channel-group) and binarize with ACT Sign
            for n in range(b_per_core):
                for a in range(CG):
                    xt = xs_pool.tile([P, h, w], F32, tag="xstage")
                    nc.sync.dma_start(
                        out=xt[:], in_=x_d[n, a * P : (a + 1) * P, :, :]
                    )
                    nc.scalar.activation(
                        out=acts[:, a, n, 1 : h + 1, 1 : w + 1],
                        in_=xt[:],
                        func=AF.Sign,
                    )

            # ---- conv + partial BN stats ----
            ybuf = singles.tile([P, CG, NT, FREE], F32)
            sum_p = small.tile([P, CG, NT], F32)
            sumsq_p = small.tile([P, CG, NT], F32)

            for n in range(b_per_core):
                for b in range(CG):
                    for t in range(tiles_per_img):
                        r0 = t * RT
                        if use_fp8:
                            # DoubleRow: contract both channel groups at once.
                            # Moving AP must be flat 3D [K, 2, N]: take
                            # contiguous 8x64 row-blocks (8 garbage cols per
                            # row, dropped at eviction).
                            ps = psum_pool.tile([P, RT * WPAD], F32, tag="ps")
                            flat = acts[:, :, n, :, :].rearrange(
                                "p g h w -> p g (h w)"
                            )
                            k = 0
                            for kh in range(3):
                                for kw in range(3):
                                    st = (r0 + kh) * WPAD + kw
                                    nc.tensor.matmul(
                                        ps[:],
                                        lhsT=wsb[
                                            :, :, kh * 3 + kw, b * P : (b + 1) * P
                                        ],
                                        rhs=flat[:, :, st : st + RT * WPAD],
                                        start=(k == 0),
                                        stop=(k == 8),
                                        perf_mode=mybir.MatmulPerfMode.DoubleRow,
                                    )
                                    k += 1
                        else:
                            ps = psum_pool.tile([P, FREE], F32, tag="ps")
                            k = 0
                            for a in range(CG):
                                for kh in range(3):
                                    for kw in range(3):
                                        nc.tensor.matmul(
                                            ps[:],
                                            lhsT=wsb[
                                                :, a, kh * 3 + kw,
                                                b * P : (b + 1) * P,
                                            ],
                                            rhs=acts[
                                                :, a, n, r0 + kh : r0 + kh + RT,
                                                kw : kw + w,
                                            ],
                                            start=(k == 0),
                                            stop=(k == 17),
                                        )
                                        k += 1
                        idx = n * tiles_per_img + t
                        if use_fp8:
                            ps_v = ps[:].rearrange(
                                "p (r c) -> p r c", r=RT
                            )[:, :, 0:w]
                        else:
                            ps_v = ps[:]
                        # evict: copy PSUM->SBUF + per-channel sum (VectorE;
                        # single PSUM input — PSUM has one DVE read port)
                        nc.vector.tensor_scalar(
                            out=ybuf[:, b, idx, :],
                            in0=ps_v,
                            scalar1=0.0,
                            scalar2=None,
                            op0=add,
                            op1=add,
                            accum_out=sum_p[:, b, idx : idx + 1],
                        )
                        # square + per-channel sumsq (ScalarE)
                        sqt = sq_pool.tile([P, FREE], F32, tag="sq")
                        nc.scalar.activation(
                            out=sqt[:],
                            in_=ps_v,
                            func=AF.Square,
                            accum_out=sumsq_p[:, b, idx : idx + 1],
                        )

            # ---- reduce partials, AllReduce across cores ----
            stats = small.tile([P, 4], F32)
            nc.vector.tensor_reduce(
                out=stats[:, 0:2], in_=sum_p[:], axis=mybir.AxisListType.X, op=add
            )
            nc.vector.tensor_reduce(
                out=stats[:, 2:4], in_=sumsq_p[:], axis=mybir.AxisListType.X, op=add
            )
            # AllGather (lower floor than AllReduce for 2KB) + local reduce
            in_bounce = dram.tile([P, 4], F32)
            out_bounce = dram.tile([n_cores * P, 4], F32)
            nc.gpsimd.dma_start(out=in_bounce[:], in_=stats[:])
            nc.gpsimd.collective_compute(
                "AllGather",
                mybir.AluOpType.bypass,
                replica_groups=[list(range(n_cores))],
                ins=[in_bounce.opt()],
                outs=[out_bounce.opt()],
            )
            gst8 = small.tile([P, 4, n_cores], F32)
            nc.gpsimd.dma_start(
                out=gst8[:],
                in_=out_bounce[:].rearrange("(c p) s -> p s c", c=n_cores),
            )
            gstats = small.tile([P, 4], F32)
            nc.vector.tensor_reduce(
                out=gstats[:], in_=gst8[:], axis=mybir.AxisListType.X, op=add
            )

            # ---- per-channel affine coefficients ----
            # mean = sum/nhw ; ex2 = sumsq/nhw ; var_y = (ex2 - mean^2)*sw^2
            # rstd = 1/sqrt(var_y + eps) ; a = gamma*sw*rstd ; b = beta - mean*a
            mean_t = small.tile([P, CG], F32)
            nc.vector.tensor_scalar_mul(mean_t[:], gstats[:, 0:2], 1.0 / nhw_total)
            ex2_t = small.tile([P, CG], F32)
            nc.vector.tensor_scalar_mul(ex2_t[:], gstats[:, 2:4], 1.0 / nhw_total)
            m2_t = small.tile([P, CG], F32)
            nc.vector.tensor_tensor(
                out=m2_t[:], in0=mean_t[:], in1=mean_t[:], op=mult
            )
            var_t = small.tile([P, CG], F32)
            nc.vector.tensor_tensor(
                out=var_t[:], in0=ex2_t[:], in1=m2_t[:], op=subtract
            )
            nc.vector.tensor_tensor(
                out=var_t[:], in0=var_t[:], in1=coef[:, :, 2], op=mult
            )
            eps_t = small.tile([P, 1], F32)
            nc.vector.memset(eps_t[:], BN_EPS)
            std_t = small.tile([P, CG], F32)
            nc.scalar.activation(
                out=std_t[:], in_=var_t[:], func=AF.Sqrt, bias=eps_t[:], scale=1.0
            )
            rstd_t = small.tile([P, CG], F32)
            nc.vector.reciprocal(out=rstd_t[:], in_=std_t[:])
            a_t = small.tile([P, CG], F32)
            nc.vector.tensor_tensor(
                out=a_t[:], in0=coef[:, :, 0], in1=rstd_t[:], op=mult
            )
            ma_t = small.tile([P, CG], F32)
            nc.vector.tensor_tensor(
                out=ma_t[:], in0=mean_t[:], in1=a_t[:], op=mult
            )
            b_t = small.tile([P, CG], F32)
            nc.vector.tensor_tensor(
                out=b_t[:], in0=coef[:, :, 1], in1=ma_t[:], op=subtract
            )

            # ---- apply affine + hardtanh, stream out ----
            # chunks of CH tiles per op to amortize per-op overhead; the DMA
            # out (12.8 MB) is the tail's floor and pipelines with ACT/DVE
            for b in range(CG):
                for n in range(b_per_core):
                    t = 0
                    while t < tiles_per_img:
                        ch = min(2, tiles_per_img - t)
                        idx = n * tiles_per_img + t
                        st = stage_pool.tile([P, 2 * FREE], F32, tag="aff")
                        nc.scalar.activation(
                            out=st[:, 0 : ch * FREE],
                            in_=ybuf[:, b, idx : idx + ch, :],
                            func=AF.Identity,
                            bias=b_t[:, b : b + 1],
                            scale=a_t[:, b : b + 1],
                        )
                        st2 = stage_pool.tile([P, 2 * FREE], F32, tag="clip")
                        nc.vector.tensor_scalar(
                            out=st2[:, 0 : ch * FREE],
                            in0=st[:, 0 : ch * FREE],
                            scalar1=1.0,
                            scalar2=-1.0,
                            op0=amin,
                            op1=amax,
                        )
                        nc.sync.dma_start(
                            out=out_d[
                                n,
                                b * P : (b + 1) * P,
                                t * RT : (t + ch) * RT,
                                :,
                            ],
                            in_=st2[:, 0 : ch * FREE],
                        )
                        t += ch

    nc.compile()
    return nc


def prep_inputs(x, weight, gamma, beta, b_per_core, n_cores, use_fp8=True):
    """Host-side prep: weight standardization/sign/scale + sharding."""
    w64 = np.asarray(weight, dtype=np.float64)
    co = w64.shape[0]
    wf = w64.reshape(co, -1)
    mean = wf.mean(axis=1)
    bw = w64 - mean[:, None, None, None]
    std = bw.reshape(co, -1).std(axis=1, ddof=1)
    mb = np.abs(bw / std[:, None, None, None]).reshape(co, -1).mean(axis=1)
    sw = 2.0 ** np.round(np.log2(mb))
    sgn = np.sign(bw)  # {-1, 0, +1}

    # wsgn[p, a, t, co] = sgn[co, a*128+p, kh, kw]
    s = sgn.reshape(co, CG, P, 9)
    wsgn = np.ascontiguousarray(s.transpose(2, 1, 3, 0))
    adt_np = ml_dtypes.float8_e4m3 if use_fp8 else ml_dtypes.bfloat16
    wsgn = wsgn.astype(adt_np)

    ga = (np.asarray(gamma, dtype=np.float64) * sw).astype(np.float32)
    be = np.asarray(beta, dtype=np.float32)
    sw2 = (sw * sw).astype(np.float32)
    coef = np.stack(
        [
            ga.reshape(CG, P).T,       # [p, g]
            be.reshape(CG, P).T,
            sw2.reshape(CG, P).T,
        ],
        axis=-1,
    ).astype(np.float32)               # [P, CG, 3]

    x = np.asarray(x, dtype=np.float32)
    in_maps = []
    for c in range(n_cores):
        in_maps.append(
            {
                "x": np.ascontiguousarray(
                    x[c * b_per_core : (c + 1) * b_per_core]
                ),
                "wsgn": wsgn,
                "coef": coef,
            }
        )
    return in_maps


_CACHE = {}


def _get_nc(key, **kw):
    if key not in _CACHE:
        _CACHE[key] = build_kernel(**kw)
    return _CACHE[key]


def run(x, weight, gamma, beta, use_fp8=True, trace=False):
    n, c, h, w = x.shape
    b_per_core = n // N_CORES
    nc = _get_nc(
        (b_per_core, h, w, use_fp8),
        b_per_core=b_per_core,
        h=h,
        w=w,
        n_cores=N_CORES,
        use_fp8=use_fp8,
    )
    in_maps = prep_inputs(
        x, weight, gamma, beta, b_per_core, N_CORES, use_fp8=use_fp8
    )
    res = run_bass_kernel_spmd(nc, in_maps, list(range(N_CORES)), trace=trace)
    out = np.concatenate([r["out"] for r in res.results], axis=0)
    return out, res


def kernel(x, weight, gamma, beta):
    out, _ = run(x, weight, gamma, beta, use_fp8=True)
    return out
